# revision 2
# baseline (speedup 1.0000x reference)
"""AttentionWithRotaryPositionalEmbedding — Trainium2 Bass kernel.

Shapes (hardcoded, from the problem spec):
  x: (8, 2048, 512), mask: (8, 2048), W_qkv: (1536, 512),
  W_proj: (512, 512), b_proj: (512,), num_cls_token: scalar
Sharding: data-parallel over batch B=8 across the 8 NeuronCores; weights
replicated. No collectives.

Per-core dataflow (batch b):
  qkv = x_b @ W_qkv^T           via PE (fp32r), xT stationary
  RoPE on q,k                   DVE reads qkv PSUM directly (fused evac)
  q,k -> [d, n] layout          PE transpose
  per head-pair (2 heads packed in the 128x128 array via row tiling):
    scoresT[m, n] = k^T q       PE, K=64 per head
    eT = exp(0.125*scoresT + mask[m])   ScalarE, mask as per-partition bias
    num[d|den, n] += v_aug^T eT  PE, ones-column of v_aug yields softmax
                                 denominators in row 64 for free
    attnT = num[0:64] * (1/den)  DVE + gpsimd partition_broadcast
  outT = W_proj attnT + b_proj  PE + DVE bias, stored transposed (host
                                transposes back)
"""

import numpy as np

import concourse.bass as bass
import concourse.tile as tile
from concourse import bacc, mybir
from concourse.alu_op_type import AluOpType
from concourse.masks import make_identity

P = 128
B = 8
N = 2048
C = 512
H = 8
D = 64
F = 3 * C          # 1536
NB = N // P        # 16 token blocks
CB = C // P        # 4 contraction chunks
PAIRS = H // 2     # 4 head pairs
NCH = N // 512     # 4 query chunks of 512
VW = H * (D + 1)   # 520: per head [v_h (64) | 1.0] -> AV matmul M=65 rows,
                   # row 64 = softmax denominator

F32 = mybir.dt.float32
F32R = mybir.dt.float32r
EXP = mybir.ActivationFunctionType.Exp


def build_nc(repeats: int = 1, debug: bool = False):
    nc = bacc.Bacc("TRN2", target_bir_lowering=False, debug=False, num_devices=B)

    # All inputs pre-shuffled on the host so every DMA row is contiguous
    # (strided gathers explode into per-element descriptors and overflow the
    # 16-bit DMA-queue semaphores).
    xT = nc.dram_tensor("xT", [NB, P, CB, P], F32R, kind="ExternalInput").ap()
    wqkvT = nc.dram_tensor("wqkvT", [P, CB, F], F32R, kind="ExternalInput").ap()
    wprojT = nc.dram_tensor("wprojT", [P, CB, C], F32R, kind="ExternalInput").ap()
    bproj = nc.dram_tensor("bproj", [P, CB], F32, kind="ExternalInput").ap()
    maskd = nc.dram_tensor("maskd", [P, NB], F32, kind="ExternalInput").ap()
    cosd = nc.dram_tensor("cosd", [P, NB, D // 2], F32, kind="ExternalInput").ap()
    sind = nc.dram_tensor("sind", [P, NB, D // 2], F32, kind="ExternalInput").ap()
    outT = nc.dram_tensor("outT", [C, N], F32, kind="ExternalOutput").ap()
    dbg = {}
    if debug:
        dbg["qT"] = nc.dram_tensor("dbg_qT", [P, PAIRS, N], F32,
                                   kind="ExternalOutput").ap()
        dbg["kT"] = nc.dram_tensor("dbg_kT", [P, PAIRS, N], F32,
                                   kind="ExternalOutput").ap()
        dbg["vaug"] = nc.dram_tensor("dbg_vaug", [P, NB, VW], F32,
                                     kind="ExternalOutput").ap()
        dbg["attnT"] = nc.dram_tensor("dbg_attnT", [P, CB, N], F32,
                                      kind="ExternalOutput").ap()

    with tile.TileContext(nc) as tc:
        with (
            tc.tile_pool(name="singles", bufs=1) as singles,
            tc.tile_pool(name="rope", bufs=3) as rope_pool,
            tc.tile_pool(name="qkrop", bufs=3) as qkrop_pool,
            tc.tile_pool(name="eT", bufs=3) as eT_pool,
            tc.tile_pool(name="den", bufs=2) as den_pool,
            tc.tile_pool(name="outp", bufs=2) as out_pool,
            tc.tile_pool(name="dram", bufs=2, space="DRAM") as dram_pool,
            tc.tile_pool(name="misc_ps", bufs=2, space="PSUM") as misc_ps,
            tc.tile_pool(name="pair_ps", bufs=2, space="PSUM") as pair_ps,
            tc.tile_pool(name="num_ps", bufs=2, space="PSUM") as num_ps,
        ):
            # ---- resident inputs ----
            wqkvT_sb = singles.tile([P, CB, F], F32R)
            nc.sync.dma_start(out=wqkvT_sb, in_=wqkvT)
            wprojT_sb = singles.tile([P, CB, C], F32R)
            nc.sync.dma_start(out=wprojT_sb, in_=wprojT)
            bproj_sb = singles.tile([P, CB], F32)
            nc.sync.dma_start(out=bproj_sb, in_=bproj)
            mask_sb = singles.tile([P, NB], F32)
            nc.sync.dma_start(out=mask_sb, in_=maskd)
            cos_sb = singles.tile([P, NB, D // 2], F32)
            nc.sync.dma_start(out=cos_sb, in_=cosd)
            sin_sb = singles.tile([P, NB, D // 2], F32)
            nc.sync.dma_start(out=sin_sb, in_=sind)
            ident = singles.tile([P, P], F32)
            make_identity(nc, ident)
            identr_t = singles.tile([P, P], F32R)
            nc.vector.tensor_copy(out=identr_t, in_=ident)
            identr = identr_t[:]
            ones8 = singles.tile([P, H], F32)
            nc.vector.memset(ones8, 1.0)

            # ---- resident intermediates ----
            qT_sb = singles.tile([P, PAIRS, N], F32R)
            kT_sb = singles.tile([P, PAIRS, N], F32R)
            vaug_sb = singles.tile([P, NB, VW], F32R)

            def rope_block(psum_qk, i, dst):
                """Rotate q/k block [128, 512] from PSUM into SBUF tile dst."""
                v4 = psum_qk[:].rearrange("p (h j t) -> p h j t", h=H, t=2)
                un = v4[:, :, :, 0:1].squeeze(3)
                ev = v4[:, :, :, 1:2].squeeze(3)
                cosb = cos_sb[:, i, :].unsqueeze(1).broadcast_to([P, H, D // 2])
                sinb = sin_sb[:, i, :].unsqueeze(1).broadcast_to([P, H, D // 2])
                d4 = dst[:].rearrange("p (h j t) -> p h j t", h=H, t=2)
                d_un = d4[:, :, :, 0:1].squeeze(3)
                d_ev = d4[:, :, :, 1:2].squeeze(3)
                t1 = rope_pool.tile([P, H, D // 2], F32, tag="ropetmp")
                t2 = rope_pool.tile([P, H, D // 2], F32, tag="ropetmp")
                nc.vector.tensor_tensor(out=t1, in0=un, in1=cosb, op=AluOpType.mult)
                nc.vector.tensor_tensor(out=t2, in0=ev, in1=sinb, op=AluOpType.mult)
                nc.vector.tensor_sub(out=d_un, in0=t1[:], in1=t2[:])
                t3 = rope_pool.tile([P, H, D // 2], F32, tag="ropetmp")
                t4 = rope_pool.tile([P, H, D // 2], F32, tag="ropetmp")
                nc.vector.tensor_tensor(out=t3, in0=un, in1=sinb, op=AluOpType.mult)
                nc.vector.tensor_tensor(out=t4, in0=ev, in1=cosb, op=AluOpType.mult)
                nc.vector.tensor_add(out=d_ev, in0=t3[:], in1=t4[:])

            def body():
                # ================= stage A: qkv + rope + transpose ============
                for i in range(NB):
                    nsl = bass.ts(i, P)
                    xblk = qkrop_pool.tile([P, CB, P], F32R, tag="xblk")
                    nc.sync.dma_start(out=xblk, in_=xT[i])
                    # --- q ---
                    ps_q = misc_ps.tile([P, 512], F32, tag="misc")
                    for cc in range(CB):
                        nc.tensor.matmul(ps_q[:], lhsT=xblk[:, cc, :],
                                         rhs=wqkvT_sb[:, cc, 0:512],
                                         start=(cc == 0), stop=(cc == CB - 1))
                    q_rot = qkrop_pool.tile([P, 512], F32R, tag="qk")
                    rope_block(ps_q, i, q_rot)
                    # --- k ---
                    ps_k = misc_ps.tile([P, 512], F32, tag="misc")
                    for cc in range(CB):
                        nc.tensor.matmul(ps_k[:], lhsT=xblk[:, cc, :],
                                         rhs=wqkvT_sb[:, cc, 512:1024],
                                         start=(cc == 0), stop=(cc == CB - 1))
                    k_rot = qkrop_pool.tile([P, 512], F32R, tag="qk")
                    rope_block(ps_k, i, k_rot)
                    # --- v ---
                    ps_v = misc_ps.tile([P, 512], F32, tag="misc")
                    for cc in range(CB):
                        nc.tensor.matmul(ps_v[:], lhsT=xblk[:, cc, :],
                                         rhs=wqkvT_sb[:, cc, 1024:1536],
                                         start=(cc == 0), stop=(cc == CB - 1))
                    vrow = vaug_sb[:, i, :].rearrange("p (h e) -> p h e", e=D + 1)
                    nc.vector.tensor_copy(out=vrow[:, :, D:D + 1],
                                          in_=ones8[:].unsqueeze(2))
                    nc.vector.tensor_copy(
                        out=vrow[:, :, 0:D],
                        in_=ps_v[:].rearrange("p (h d) -> p h d", h=H))
                    # --- transposes: [n, f] -> [f, n] ---
                    for j in range(PAIRS):
                        fsl = bass.ts(j, P)
                        tq = misc_ps.tile([P, P], F32R, tag="misc")
                        nc.tensor.transpose(tq[:], q_rot[:, fsl], identr)
                        nc.vector.tensor_copy(out=qT_sb[:, j, nsl], in_=tq[:])
                        tk = misc_ps.tile([P, P], F32R, tag="misc")
                        nc.tensor.transpose(tk[:], k_rot[:, fsl], identr)
                        nc.vector.tensor_copy(out=kT_sb[:, j, nsl], in_=tk[:])

                # ================= stage B: attention =========================
                for ncq in range(NCH):
                    qsl = bass.ts(ncq, 512)
                    attn_t = out_pool.tile([P, CB, 512], F32R, tag="attn")
                    for pj in range(PAIRS):
                        hA, hB = 2 * pj, 2 * pj + 1
                        num_A = num_ps.tile([P, 512], F32, tag="num")
                        num_B = num_ps.tile([P, 512], F32, tag="num")
                        for mb in range(NB):
                            msl = bass.ts(mb, P)
                            pairt = pair_ps.tile([P, 1024], F32, tag="pair")
                            nc.tensor.matmul(pairt[:, 0:512],
                                             lhsT=kT_sb[0:64, pj, msl],
                                             rhs=qT_sb[0:64, pj, qsl],
                                             start=True, stop=True,
                                             tile_position=(0, 0))
                            nc.tensor.matmul(pairt[:, 512:1024],
                                             lhsT=kT_sb[64:128, pj, msl],
                                             rhs=qT_sb[64:128, pj, qsl],
                                             start=True, stop=True,
                                             tile_position=(64, 0))
                            eT = eT_pool.tile([P, 1024], F32R, tag="eT")
                            nc.scalar.activation(out=eT[:], in_=pairt[:], func=EXP,
                                                 bias=mask_sb[:, mb:mb + 1],
                                                 scale=0.125)
                            nc.tensor.matmul(
                                num_A[0:D + 1, :],
                                lhsT=vaug_sb[:, mb, hA * 65:(hA + 1) * 65],
                                rhs=eT[:, 0:512],
                                start=(mb == 0), stop=(mb == NB - 1))
                            nc.tensor.matmul(
                                num_B[0:D + 1, :],
                                lhsT=vaug_sb[:, mb, hB * 65:(hB + 1) * 65],
                                rhs=eT[:, 512:1024],
                                start=(mb == 0), stop=(mb == NB - 1))
                        # num rows 0:64 = sum(e*v); row 64 = denominator.
                        # Broadcast 1/den across 64 partitions via SBUF->SBUF
                        # DMA, then normalize. attnT head A -> partitions
                        # 0:64, head B -> 64:128 of c-block pj.
                        rc = den_pool.tile([33, 512], F32, tag="recip")
                        den = den_pool.tile([P, 512], F32, tag="den")
                        nc.vector.reciprocal(out=rc[0:1, :], in_=num_A[64:65, :])
                        nc.vector.reciprocal(out=rc[32:33, :], in_=num_B[64:65, :])
                        scr = dram_pool.tile([2, 512], F32, tag="scr")
                        nc.sync.dma_start(out=scr[0:1, :], in_=rc[0:1, :])
                        nc.sync.dma_start(out=scr[1:2, :], in_=rc[32:33, :])
                        nc.gpsimd.dma_start(
                            out=den[0:64, :],
                            in_=scr[0:1, :].squeeze(0).partition_broadcast(64))
                        nc.gpsimd.dma_start(
                            out=den[64:128, :],
                            in_=scr[1:2, :].squeeze(0).partition_broadcast(64))
                        nc.vector.tensor_tensor(out=attn_t[0:64, pj, :],
                                                in0=num_A[0:64, :],
                                                in1=den[0:64, :],
                                                op=AluOpType.mult)
                        nc.vector.tensor_tensor(out=attn_t[64:128, pj, :],
                                                in0=num_B[0:64, :],
                                                in1=den[64:128, :],
                                                op=AluOpType.mult)
                    if debug:
                        nc.sync.dma_start(out=dbg["attnT"][:, :, qsl],
                                          in_=attn_t[:].bitcast(F32))
                    # ============= stage C: output projection for this ncq ====
                    for ob in range(CB):
                        po = misc_ps.tile([P, 512], F32, tag="misc")
                        for cc in range(CB):
                            nc.tensor.matmul(po[:],
                                             lhsT=wprojT_sb[:, cc, bass.ts(ob, P)],
                                             rhs=attn_t[:, cc, :],
                                             start=(cc == 0), stop=(cc == CB - 1))
                        ot = out_pool.tile([P, 512], F32, tag="out")
                        nc.vector.tensor_scalar(out=ot[:], in0=po[:],
                                                scalar1=bproj_sb[:, ob:ob + 1],
                                                scalar2=None, op0=AluOpType.add)
                        nc.sync.dma_start(out=outT[bass.ts(ob, P), qsl], in_=ot[:])

            if repeats > 1:
                with tc.For_i(0, repeats) as _i:
                    body()
            else:
                body()

            if debug:
                nc.sync.dma_start(out=dbg["qT"], in_=qT_sb[:].bitcast(F32))
                nc.sync.dma_start(out=dbg["kT"], in_=kT_sb[:].bitcast(F32))
                nc.sync.dma_start(out=dbg["vaug"], in_=vaug_sb[:].bitcast(F32))

    nc.compile()
    return nc


def prep_in_maps(x, mask, W_qkv, W_proj, b_proj, num_cls_token):
    x = np.asarray(x, dtype=np.float32)
    mask = np.asarray(mask, dtype=np.float32)
    W_qkv = np.asarray(W_qkv, dtype=np.float32)
    W_proj = np.asarray(W_proj, dtype=np.float32)
    b_proj = np.asarray(b_proj, dtype=np.float32)
    ncls = int(np.asarray(num_cls_token))

    wqkvT = np.ascontiguousarray(W_qkv.T)
    wprojT = np.ascontiguousarray(W_proj.T)

    cos = np.ones((N, D // 2), dtype=np.float32)
    sin = np.zeros((N, D // 2), dtype=np.float32)
    if ncls < N:
        inv_freq = (1.0 / (10000.0 ** (np.arange(0, D, 2, dtype=np.float32)
                                       / np.float32(D)))).astype(np.float32)
        pos = np.arange(N - ncls, dtype=np.float32)
        freqs = pos[:, None] * inv_freq[None, :]
        cos[ncls:] = np.cos(freqs).astype(np.float32)
        sin[ncls:] = np.sin(freqs).astype(np.float32)

    def part_major(a):
        # (C, X...) -> (P, CB, X...): row c = cc*128 + p  ->  [p, cc, ...]
        return np.ascontiguousarray(
            a.reshape(CB, P, *a.shape[1:]).transpose(
                1, 0, *range(2, a.ndim + 1)))

    def tok_major(a):
        # (N, X...) -> (P, NB, X...): token n = i*128 + p  ->  [p, i, ...]
        return np.ascontiguousarray(
            a.reshape(NB, P, *a.shape[1:]).transpose(
                1, 0, *range(2, a.ndim + 1)))

    wqkvT_h = part_major(wqkvT)
    wprojT_h = part_major(wprojT)
    bproj_h = part_major(b_proj)
    cos_h = tok_major(cos)
    sin_h = tok_major(sin)

    in_maps = []
    for b in range(B):
        # xT block layout [i, p, cc, j] = x[b][i*128 + j, cc*128 + p]
        xb = x[b].reshape(NB, P, CB, P).transpose(0, 3, 2, 1)
        in_maps.append({
            "xT": np.ascontiguousarray(xb),
            "wqkvT": wqkvT_h,
            "wprojT": wprojT_h,
            "bproj": bproj_h,
            "maskd": tok_major(mask[b]),
            "cosd": cos_h,
            "sind": sin_h,
        })
    return in_maps


def gather_out(results):
    out = np.empty((B, N, C), dtype=np.float32)
    for b in range(B):
        out[b] = results[b]["outT"].T
    return out


_NC_CACHE = {}

TRACE = False          # test.py sets True to capture NTFF profile + exec time
LAST_RES = None        # BassKernelResults of the last kernel() call


def kernel(**inputs) -> np.ndarray:
    global LAST_RES
    from concourse.bass_utils import run_bass_kernel_spmd
    key = "single"
    if key not in _NC_CACHE:
        _NC_CACHE[key] = build_nc(repeats=1)
    nc = _NC_CACHE[key]
    in_maps = prep_in_maps(**inputs)
    res = run_bass_kernel_spmd(nc, in_maps, list(range(B)), trace=TRACE)
    LAST_RES = res
    return gather_out(res.results)



# revision 18
# speedup vs baseline: 1.0720x; 1.0720x over previous
"""AttentionWithRotaryPositionalEmbedding — Trainium2 Bass kernel (v2).

Shapes (hardcoded, from the problem spec):
  x: (8, 2048, 512), mask: (8, 2048), W_qkv: (1536, 512),
  W_proj: (512, 512), b_proj: (512,), num_cls_token: scalar
Sharding: data-parallel over batch B=8 across the 8 NeuronCores; weights
replicated. No collectives.

Per-core dataflow (batch b):
  Stage A (k/v):  k = x_b @ W_k^T via PE (W_k host-permuted so each head is
    [un(32)|ev(32)] and pre-scaled by 2^23/ln2 * 0.125 for the softmax
    bit-trick); rope as 3 full-width DVE ops (mult, mult-on-swapped-view,
    add); PE transpose to kT[d, n]; v copied into an augmented [v|1] tile
    whose ones-column yields softmax denominators from the AV matmul.
  Per query chunk (512): same for q, then attention:
    scoresT[m, n] = k^T q    PE, two heads packed via row tiles (0,0)/(64,0)
    eT = exp(..)             key blocks 0..9: ScalarE exp (scale=1/A,
                             bias=mask); blocks 10..15: GPSIMD Schraudolph
                             (int32 add of A*mask+B, bitcast to fp32)
    num[d|den, n] += vaug^T eT   PE accumulate
    attnT = num * (1/den)    DVE reciprocal + gpsimd partition_broadcast
  outT = W_proj attnT + b_proj   PE + DVE bias (stored transposed; host
                                 transposes back)
"""

import numpy as np

import concourse.bass as bass
import concourse.tile as tile
from concourse import bacc, mybir
from concourse.alu_op_type import AluOpType
from concourse.masks import make_identity

P = 128
B = 8
N = 2048
C = 512
H = 8
D = 64
F = 3 * C          # 1536
NB = N // P        # 16 token blocks
CB = C // P        # 4 contraction chunks
PAIRS = H // 2     # 4 head pairs
NCH = N // 512     # 4 query chunks of 512
VW = H * (D + 1)   # 520

A_SCHR = float(2 ** 7) / float(np.log(2.0))    # bf16-space Schraudolph slope
C_SCHR = 366393.0 / 65536.0
B_SCHR = 127.0 * 2 ** 7 - C_SCHR
S_ACT = 12          # key blocks [0, S_ACT) use ScalarE exp; rest DVE Schraudolph

F32 = mybir.dt.float32
F32R = mybir.dt.float32r
BF16 = mybir.dt.bfloat16
I16 = mybir.dt.int16
EXP = mybir.ActivationFunctionType.Exp


def build_nc(repeats: int = 1, debug: bool = False, ablate: frozenset = frozenset()):
    """ablate: 'no_schr' -> all-ScalarE exp (error attribution);
    'no_b' / 'no_c' -> skip stages (timing only)."""
    s_act = NB if "no_schr" in ablate else S_ACT
    nc = bacc.Bacc("TRN2", target_bir_lowering=False, debug=False, num_devices=B)

    xT = nc.dram_tensor("xT", [NB, P, CB, P], F32R, kind="ExternalInput").ap()
    wqkvT = nc.dram_tensor("wqkvT", [P, CB, F], F32R, kind="ExternalInput").ap()
    wprojT = nc.dram_tensor("wprojT", [P, CB, C], F32R, kind="ExternalInput").ap()
    bproj = nc.dram_tensor("bproj", [P, CB], F32, kind="ExternalInput").ap()
    maskd = nc.dram_tensor("maskd", [P, NB], F32, kind="ExternalInput").ap()
    cosd = nc.dram_tensor("cosd", [P, NB, D // 2], F32, kind="ExternalInput").ap()
    sind = nc.dram_tensor("sind", [P, NB, D], F32, kind="ExternalInput").ap()
    outT = nc.dram_tensor("outT", [C, N], F32, kind="ExternalOutput").ap()

    with tile.TileContext(nc) as tc:
        with (
            tc.tile_pool(name="singles", bufs=1) as singles,
            tc.tile_pool(name="xblk", bufs=3) as xblk_pool,
            tc.tile_pool(name="rope", bufs=3) as rope_pool,
            tc.tile_pool(name="eT", bufs=3) as eT_pool,
            tc.tile_pool(name="den", bufs=2) as den_pool,
            tc.tile_pool(name="outp", bufs=2) as out_pool,
            tc.tile_pool(name="qT", bufs=2) as qT_pool,
            tc.tile_pool(name="misc_ps", bufs=2, space="PSUM") as misc_ps,
            tc.tile_pool(name="pair_ps", bufs=2, space="PSUM") as pair_ps,
            tc.tile_pool(name="num_ps", bufs=2, space="PSUM") as num_ps,
        ):
            # ---- resident inputs ----
            wqkvT_sb = singles.tile([P, CB, F], F32R)
            nc.sync.dma_start(out=wqkvT_sb, in_=wqkvT)
            wprojT_sb = singles.tile([P, CB, C], F32R)
            nc.sync.dma_start(out=wprojT_sb, in_=wprojT)
            bproj_sb = singles.tile([P, CB], F32)
            nc.sync.dma_start(out=bproj_sb, in_=bproj)
            mask_sb = singles.tile([P, NB], F32)
            nc.sync.dma_start(out=mask_sb, in_=maskd)
            cos_sb = singles.tile([P, NB, D // 2], F32)
            nc.sync.dma_start(out=cos_sb, in_=cosd)
            sin_sb = singles.tile([P, NB, D], F32)   # [-sin | +sin]
            nc.sync.dma_start(out=sin_sb, in_=sind)
            ident = singles.tile([P, P], F32)
            make_identity(nc, ident)
            identr_t = singles.tile([P, P], F32R)
            nc.vector.tensor_copy(out=identr_t, in_=ident)
            identr = identr_t[:]

            # ---- resident intermediates ----
            kT_sb = singles.tile([P, PAIRS, N], F32R)
            vaug_sb = singles.tile([P, NB, VW], BF16)
            # ones-columns of vaug (never overwritten by the v copies)
            vaug4 = vaug_sb[:].rearrange("p i (h e) -> p i h e", e=D + 1)
            nc.vector.memset(vaug4[:, :, :, D:D + 1], 1.0)
            # Schraudolph per-key bias: A*mask + B
            bprime = singles.tile([P, NB], F32)
            nc.vector.tensor_scalar(out=bprime, in0=mask_sb[:],
                                    scalar1=A_SCHR, scalar2=B_SCHR,
                                    op0=AluOpType.mult, op1=AluOpType.add)

            def rope_block(ps, i, dst):
                """Rotate block [128, 512] from PSUM ps into SBUF tile dst.

                Feature layout per head: [un(32) | ev(32)] (host W permute).
                r = t*[cos|cos] + swap(t)*[-sin|+sin]
                """
                t4 = ps[:].rearrange("p (h s j) -> p h s j", h=H, s=2)
                cosb = (cos_sb[:, i, :].unsqueeze(1).unsqueeze(2)
                        .broadcast_to([P, H, 2, D // 2]))
                sinb = (sin_sb[:, i, :].rearrange("p (s j) -> p s j", s=2)
                        .unsqueeze(1).broadcast_to([P, H, 2, D // 2]))
                m1 = rope_pool.tile([P, C], F32, tag="ropetmp")
                m2 = rope_pool.tile([P, C], F32, tag="ropetmp")
                m14 = m1[:].rearrange("p (h s j) -> p h s j", h=H, s=2)
                m24 = m2[:].rearrange("p (h s j) -> p h s j", h=H, s=2)
                nc.vector.tensor_tensor(out=m14, in0=t4, in1=cosb,
                                        op=AluOpType.mult)
                nc.vector.tensor_tensor(out=m24, in0=t4[:, :, ::-1, :],
                                        in1=sinb, op=AluOpType.mult)
                # combine on gpsimd (SBUF-only op) to unload DVE
                nc.gpsimd.tensor_tensor(out=dst[:], in0=m1[:], in1=m2[:],
                                        op=AluOpType.add)

            def qk_block(i, col0, dstT, dst_nsl):
                """Project+rope+transpose token block i into dstT[:, :, dst_nsl].

                col0: 0 for q, 512 for k (W_qkv column offset)."""
                xblk = xblk_pool.tile([P, CB, P], F32R, tag="xblk")
                nc.sync.dma_start(out=xblk, in_=xT[i])
                ps = misc_ps.tile([P, C], F32, tag="misc")
                for cc in range(CB):
                    nc.tensor.matmul(ps[:], lhsT=xblk[:, cc, :],
                                     rhs=wqkvT_sb[:, cc, col0:col0 + C],
                                     start=(cc == 0), stop=(cc == CB - 1))
                rot = rope_pool.tile([P, C], F32R, tag="qk")
                rope_block(ps, i, rot)
                tp = misc_ps.tile([P, C], F32R, tag="misc")
                for j in range(PAIRS):
                    fsl = bass.ts(j, P)
                    nc.tensor.transpose(tp[:, fsl], rot[:, fsl], identr)
                nc.vector.tensor_copy(
                    out=dstT[:, :, dst_nsl],
                    in_=tp[:].rearrange("p (j n) -> p j n", j=PAIRS))
                return xblk

            def v_block(i, xblk):
                ps = misc_ps.tile([P, C], F32, tag="misc")
                for cc in range(CB):
                    nc.tensor.matmul(ps[:], lhsT=xblk[:, cc, :],
                                     rhs=wqkvT_sb[:, cc, 1024:1536],
                                     start=(cc == 0), stop=(cc == CB - 1))
                nc.vector.tensor_copy(
                    out=vaug4[:, i, :, 0:D],
                    in_=ps[:].rearrange("p (h d) -> p h d", h=H))

            def body():
                # ---- stage A: k + v for all token blocks ----
                for i in range(NB):
                    xblk = qk_block(i, 512, kT_sb, bass.ts(i, P))
                    v_block(i, xblk)

                if "no_b" in ablate:
                    return

                for ncq in range(NCH):
                    qsl = bass.ts(ncq, 512)
                    # ---- q for this chunk (double-buffered tile) ----
                    qT_sb = qT_pool.tile([P, PAIRS, 512], F32R, tag="qT")
                    for ib in range(4):
                        i = ncq * 4 + ib
                        qk_block(i, 0, qT_sb, bass.ts(ib, P))
                    # ---- stage B ----
                    attn_t = out_pool.tile([P, CB, 512], F32R, tag="attn")
                    for pj in range(PAIRS):
                        hA, hB = 2 * pj, 2 * pj + 1
                        num_A = num_ps.tile([P, 512], F32, tag="num")
                        num_B = num_ps.tile([P, 512], F32, tag="num")
                        for mb in range(NB):
                            pairt = pair_ps.tile([P, 1024], F32, tag="pair")
                            nc.tensor.matmul(pairt[:, 0:512],
                                             lhsT=kT_sb[0:64, pj, bass.ts(mb, P)],
                                             rhs=qT_sb[0:64, pj, :],
                                             start=True, stop=True,
                                             tile_position=(0, 0))
                            nc.tensor.matmul(pairt[:, 512:1024],
                                             lhsT=kT_sb[64:128, pj, bass.ts(mb, P)],
                                             rhs=qT_sb[64:128, pj, :],
                                             start=True, stop=True,
                                             tile_position=(64, 0))
                            eT = eT_pool.tile([P, 1024], BF16, tag="eT")
                            if mb < s_act:
                                nc.scalar.activation(
                                    out=eT[:], in_=pairt[:], func=EXP,
                                    bias=mask_sb[:, mb:mb + 1],
                                    scale=1.0 / A_SCHR)
                            else:
                                # Schraudolph exp: bitcast(int16(z + A*mask+B))
                                # on DVE (gpsimd cannot read PSUM)
                                nc.vector.tensor_scalar(
                                    out=eT[:].bitcast(I16), in0=pairt[:],
                                    scalar1=bprime[:, mb:mb + 1],
                                    scalar2=None, op0=AluOpType.add)
                            nc.tensor.matmul(
                                num_A[0:D + 1, :],
                                lhsT=vaug_sb[:, mb, hA * 65:(hA + 1) * 65],
                                rhs=eT[:, 0:512],
                                start=(mb == 0), stop=(mb == NB - 1))
                            nc.tensor.matmul(
                                num_B[0:D + 1, :],
                                lhsT=vaug_sb[:, mb, hB * 65:(hB + 1) * 65],
                                rhs=eT[:, 512:1024],
                                start=(mb == 0), stop=(mb == NB - 1))
                        rcA = den_pool.tile([1, 512], F32, tag="recip")
                        rcB = den_pool.tile([1, 512], F32, tag="recip")
                        denA = den_pool.tile([64, 512], F32, tag="den")
                        denB = den_pool.tile([64, 512], F32, tag="den")
                        nc.vector.reciprocal(out=rcA, in_=num_A[64:65, :])
                        nc.vector.reciprocal(out=rcB, in_=num_B[64:65, :])
                        nc.gpsimd.partition_broadcast(denA[:], rcA[:])
                        nc.gpsimd.partition_broadcast(denB[:], rcB[:])
                        nc.vector.tensor_tensor(out=attn_t[0:64, pj, :],
                                                in0=num_A[0:64, :],
                                                in1=denA[:],
                                                op=AluOpType.mult)
                        nc.vector.tensor_tensor(out=attn_t[64:128, pj, :],
                                                in0=num_B[0:64, :],
                                                in1=denB[:],
                                                op=AluOpType.mult)
                    # ---- stage C ----
                    if "no_c" in ablate:
                        continue
                    for ob in range(CB):
                        po = misc_ps.tile([P, 512], F32, tag="misc")
                        for cc in range(CB):
                            nc.tensor.matmul(po[:],
                                             lhsT=wprojT_sb[:, cc, bass.ts(ob, P)],
                                             rhs=attn_t[:, cc, :],
                                             start=(cc == 0), stop=(cc == CB - 1))
                        ot = out_pool.tile([P, 512], F32, tag="out")
                        nc.vector.tensor_scalar(out=ot[:], in0=po[:],
                                                scalar1=bproj_sb[:, ob:ob + 1],
                                                scalar2=None, op0=AluOpType.add)
                        nc.sync.dma_start(out=outT[bass.ts(ob, P), qsl], in_=ot[:])

            if repeats > 1:
                with tc.For_i(0, repeats) as _i:
                    body()
            else:
                body()

    nc.compile()
    return nc


def prep_in_maps(x, mask, W_qkv, W_proj, b_proj, num_cls_token):
    x = np.asarray(x, dtype=np.float32)
    mask = np.asarray(mask, dtype=np.float32)
    W_qkv = np.asarray(W_qkv, dtype=np.float32)
    W_proj = np.asarray(W_proj, dtype=np.float32)
    b_proj = np.asarray(b_proj, dtype=np.float32)
    ncls = int(np.asarray(num_cls_token))

    # De-interleave q/k head dims: within each head, rows [0,2,..,62, 1,3,..,63]
    # so rope operates on contiguous [un|ev] halves. Scores are invariant to
    # a shared q/k permutation. Scale k by A*0.125 for the Schraudolph path.
    perm64 = np.concatenate([np.arange(0, D, 2), np.arange(1, D, 2)])
    perm = np.arange(F)
    for sec in range(2):                       # q rows 0:512, k rows 512:1024
        for h in range(H):
            base = sec * C + h * D
            perm[base:base + D] = base + perm64
    Wp = W_qkv[perm]
    Wp = Wp.copy()
    Wp[C:2 * C] *= np.float32(A_SCHR * 0.125)   # fold A*scale into k
    wqkvT = np.ascontiguousarray(Wp.T)
    wprojT = np.ascontiguousarray(W_proj.T)

    # rope tables in de-interleaved order: j-th column is original dim 2j
    cos = np.ones((N, D // 2), dtype=np.float32)
    sin = np.zeros((N, D // 2), dtype=np.float32)
    if ncls < N:
        inv_freq = (1.0 / (10000.0 ** (np.arange(0, D, 2, dtype=np.float32)
                                       / np.float32(D)))).astype(np.float32)
        pos = np.arange(N - ncls, dtype=np.float32)
        freqs = pos[:, None] * inv_freq[None, :]
        cos[ncls:] = np.cos(freqs).astype(np.float32)
        sin[ncls:] = np.sin(freqs).astype(np.float32)
    sin_ext = np.concatenate([-sin, sin], axis=1)           # (N, 64)

    def part_major(a):
        return np.ascontiguousarray(
            a.reshape(CB, P, *a.shape[1:]).transpose(
                1, 0, *range(2, a.ndim + 1)))

    def tok_major(a):
        return np.ascontiguousarray(
            a.reshape(NB, P, *a.shape[1:]).transpose(
                1, 0, *range(2, a.ndim + 1)))

    wqkvT_h = part_major(wqkvT)
    wprojT_h = part_major(wprojT)
    bproj_h = part_major(b_proj)
    cos_h = tok_major(cos)
    sin_h = tok_major(sin_ext)

    in_maps = []
    for b in range(B):
        xb = x[b].reshape(NB, P, CB, P).transpose(0, 3, 2, 1)
        in_maps.append({
            "xT": np.ascontiguousarray(xb),
            "wqkvT": wqkvT_h,
            "wprojT": wprojT_h,
            "bproj": bproj_h,
            "maskd": tok_major(mask[b]),
            "cosd": cos_h,
            "sind": sin_h,
        })
    return in_maps


def gather_out(results):
    out = np.empty((B, N, C), dtype=np.float32)
    for b in range(B):
        out[b] = results[b]["outT"].T
    return out


_NC_CACHE = {}

TRACE = False          # test.py sets True to capture NTFF profile + exec time
LAST_RES = None        # BassKernelResults of the last kernel() call


def kernel(**inputs) -> np.ndarray:
    global LAST_RES
    from concourse.bass_utils import run_bass_kernel_spmd
    key = "single"
    if key not in _NC_CACHE:
        _NC_CACHE[key] = build_nc(repeats=1)
    nc = _NC_CACHE[key]
    in_maps = prep_in_maps(**inputs)
    res = run_bass_kernel_spmd(nc, in_maps, list(range(B)), trace=TRACE)
    LAST_RES = res
    return gather_out(res.results)
